# revision 54
# baseline (speedup 1.0000x reference)
"""Differential attention kernel for Trainium2 (8 NeuronCores).

Key identities: everything after the differential combine is linear, so
    out = diff_attn @ (x @ Wv) @ Wo + bias == (diff_attn @ x) @ (Wv @ Wo) + bias
W2 = Wv @ Wo is a [512, 512] weight-only product folded on the host, which
removes the 8192-wide V projection / attn@V / out_proj entirely; and the
device contracts attention with x FIRST (p@x is S-wide, then a small @W2),
so no V-like projection of the full batch is ever materialized.

Sharding: 2 batches x 4 query-quarters (512 queries per core). Each core
computes full K for its batch plus Q for its own quarter, its 512x2048
slice of both attention maps, and the 512x512 output slice. Only the K
projection is duplicated within a batch group.

GEMMs run as float32r (fp32 storage, full-rate PE mode) except the
attention map / x-row operands of the p@x stage, which are bf16 (PE
transposes at 1.0 vs 1.5 cyc/row; 256-wide q-half streams at full rate).
Softmax is computed without max-subtraction (|logits| < ~12, safe in fp32).
The 1/sum(e1) normalizer is folded into the final PSUM eviction, so the
combine is a single fused DVE op computing -p = e2*(lam*s1/s2) - e1; the sign
is absorbed by evicting with scale = -1/s1. bqkv is applied via the ScalarE
bias port; bv/bo fold exactly on the host via sum_k(diff_attn[q,:]) == 1-lam.

Schedule notes (sim-tuned): scores use 2-bank psum tiles so each exp is
1024 wide (ACT instruction count gates the scores phase); combine of qb-1
is pipelined behind scores(qb); wT = -(p@x)^T runs in two q-halves so half 0
starts before the last combine finishes; the setup pool stays open through
the main phase so the next iteration's input DMAs prefetch mid-iteration;
output DMAs and the tail-only x/lam/bq loads ride the GpSimd queue so they
never head-of-line block the next iteration's input loads on the SP queue.
"""

import math

import ml_dtypes
import numpy as np

import concourse.bass as bass
from concourse import bacc
import concourse.mybir as mybir
import concourse.tile as tile
from concourse import bass_utils
from concourse.bass import ts, ds
from concourse.masks import make_identity

# Problem shapes (hardcoded per harness contract).
B = 2
S = 2048
D = 512
DM = 512             # output dim
P = 128
QC = 512             # queries per core
NQ = S // QC         # 4 q-shards per batch
SCALE = 1.0 / math.sqrt(64.0)
LAMBDA_INIT = 0.8
LAYER_INDEX = 0

F32 = mybir.dt.float32
F32R = mybir.dt.float32r
BF16 = mybir.dt.bfloat16
EXP = mybir.ActivationFunctionType.Exp
IDENT = mybir.ActivationFunctionType.Identity
AXX = mybir.AxisListType.X
MUL = mybir.AluOpType.mult
SUB = mybir.AluOpType.subtract

KD = D // P          # 4 contraction chunks of the input dim
MQ = (2 * D) // P    # 8 bias columns (Q: 0..3, K: 4..7)
SN = S // 512        # 4 key chunks of 512
NKC = S // P         # 16 key chunks of 128
QB = QC // P         # 4 q-blocks per core



def kernel_body(tc, xT, xn, wqkv, w2, lam, bq, out):
    nc = tc.nc
    # tolerate f32-typed dram tensors (e.g. run_kernel's sim harness)
    if xT.dtype != F32R:
        xT = xT.bitcast(F32R)
    if wqkv.dtype != F32R:
        wqkv = wqkv.bitcast(F32R)
    if w2.dtype != F32R:
        w2 = w2.bitcast(F32R)

    with tc.tile_pool(name="persist", bufs=1) as persist:
        kT = persist.tile([P, 4, S], F32R)      # K1,K2 transposed: [feat, k]
        qT = persist.tile([P, 4, QC], F32R)     # Q1,Q2 own slice:  [feat, q]
        xn_sb = persist.tile([P, NKC, D], BF16) # x rows: [k-in-block, kc, d]
        w2_sb = persist.tile([P, KD, DM], F32R)
        lam_sb = persist.tile([P, 1], F32)
        bq_sb = persist.tile([P, MQ], F32)
        ident_f32 = persist.tile([P, P], F32)
        ident = persist.tile([P, P], BF16)

        nc.gpsimd.dma_start(lam_sb, lam)
        nc.gpsimd.dma_start(bq_sb, bq)
        make_identity(nc, ident_f32)
        nc.vector.tensor_copy(ident, ident_f32)

        # ---------------- setup: Q, K, V2 projections ----------------
        # the setup pool stays open through the main phase so the main-loop
        # pools get disjoint SBUF; the next iteration's setup region then
        # frees mid-iteration and its input DMAs overlap this one's attention
        with (
            tc.tile_pool(name="setup", bufs=1) as setup,
        ):
            xTs = setup.tile([P, KD, S], F32R)
            wq_sb = setup.tile([P, KD, 2 * D], F32R)

            # loads in first-consumption order (Q proj, K proj, V2); own
            # q-slice is columns 0:512 (host rotates x per core)
            for dc in range(KD):
                nc.sync.dma_start(wq_sb[:, dc, :D], wqkv[ds(dc * P, P), :D])
                nc.sync.dma_start(xTs[:, dc, ts(0, 512)], xT[ds(dc * P, P), ts(0, 512)])
            for dc in range(KD):
                nc.sync.dma_start(wq_sb[:, dc, D:], wqkv[ds(dc * P, P), D:])
            for sn in range(1, SN):
                for dc in range(KD):
                    nc.sync.dma_start(xTs[:, dc, ts(sn, 512)],
                                      xT[ds(dc * P, P), ts(sn, 512)])
            for dc in range(KD):
                nc.sync.dma_start(w2_sb[:, dc], w2[ds(dc * P, P), :])
            for kc in range(NKC):
                nc.gpsimd.dma_start(xn_sb[:, kc], xn[ds(kc * P, P), :])

            with tc.tile_pool(name="spsum", bufs=6, space="PSUM") as spsum:
                # Q proj (own slice = keys 0:512 of the rotated batch)
                for m in range(4):
                    pt = spsum.tile([P, QC], F32, tag="ps")
                    for dc in range(KD):
                        nc.tensor.matmul(pt, wq_sb[:, dc, ts(m, P)],
                                         xTs[:, dc, ts(0, 512)],
                                         start=(dc == 0), stop=(dc == KD - 1))
                    nc.scalar.activation(qT[:, m], pt, IDENT,
                                         bias=bq_sb[:, m : m + 1])
                # K proj (full batch)
                for sn in range(SN):
                    for m in range(4):
                        pt = spsum.tile([P, 512], F32, tag="ps")
                        for dc in range(KD):
                            nc.tensor.matmul(pt, wq_sb[:, dc, ts(4 + m, P)],
                                             xTs[:, dc, ts(sn, 512)],
                                             start=(dc == 0), stop=(dc == KD - 1))
                        nc.scalar.activation(kT[:, m, ts(sn, 512)], pt, IDENT,
                                             bias=bq_sb[:, 4 + m : 5 + m])

            # ---------------- main: attention for own 512 queries ----------
            with (
                tc.tile_pool(name="e1p", bufs=2) as e1p,
                tc.tile_pool(name="pdp", bufs=2) as pdp,
                tc.tile_pool(name="e2p", bufs=2) as e2p,
                tc.tile_pool(name="smallp", bufs=3) as smallp,
                tc.tile_pool(name="r1p", bufs=QB) as r1p,
                tc.tile_pool(name="ptp", bufs=1) as ptp,
                tc.tile_pool(name="ofp", bufs=4) as ofp,
                tc.tile_pool(name="wtp", bufs=1) as wtp,
                tc.tile_pool(name="wps", bufs=2, space="PSUM") as wps,
                tc.tile_pool(name="tps", bufs=2, space="PSUM") as _tps,
                tc.tile_pool(name="fps", bufs=2, space="PSUM") as fps,
            ):
                _run_attention(nc, qT, kT, xn_sb, w2_sb, lam_sb, ident, out,
                               e1p, e2p, smallp, r1p, ptp, ofp, wps, _tps, fps, pdp,
                               wtp)


def _run_attention(nc, qT, kT, xn_sb, w2_sb, lam_sb, ident, out,
                   e1p, e2p, smallp, r1p, ptp, ofp, wps, _tps, fps, pdp,
                   wtp):
            ptile = ptp.tile([P, NKC, QC], BF16)
            r1s = [None] * QB
            pend = []

            def emit_scores_half(qb, mi):
                pool = e1p if mi == 0 else e2p
                et = pool.tile([P, SN, 512], BF16, tag=f"e{mi}",
                               name=f"e{mi}_{qb}")
                st = smallp.tile([P, SN // 2], F32, tag=f"sum{mi}",
                                 name=f"sum{mi}_{qb}")
                # 2-bank psum per pair of 512-chunks -> one 1024-wide exp
                # (halves the ACT instruction count, which gates this phase)
                for knw in range(SN // 2):
                    pt = wps.tile([P, 2, 512], F32, tag="ps",
                                  name=f"ps_{qb}_{mi}_{knw}")
                    for h in range(2):
                        kn = 2 * knw + h
                        for dc in range(2):
                            nc.tensor.matmul(
                                pt[:, h],
                                qT[:, 2 * mi + dc, ts(qb, P)],
                                kT[:, 2 * mi + dc, ts(kn, 512)],
                                start=(dc == 0), stop=(dc == 1))
                    nc.scalar.activation(
                        et[:, ds(2 * knw, 2)], pt, EXP, scale=SCALE,
                        accum_out=st[:, knw : knw + 1])
                return et, st

            def emit_norms(qb, ets, sums):
                # normalizers: final evict uses -1/s1; combine uses lam*s1/s2
                s1 = smallp.tile([P, 1], F32, tag="s1", name=f"s1_{qb}")
                nc.vector.reduce_sum(s1, sums[0], axis=AXX)
                s1n = smallp.tile([P, 1], F32, tag="s1n", name=f"s1n_{qb}")
                nc.vector.tensor_scalar_mul(s1n, s1, -1.0)
                r1n = r1p.tile([P, 1], F32, tag="r1", name=f"r1_{qb}")
                nc.vector.reciprocal(r1n, s1n)
                r1s[qb] = r1n
                s2 = smallp.tile([P, 1], F32, tag="s2", name=f"s2_{qb}")
                nc.vector.reduce_sum(s2, sums[1], axis=AXX)
                r2 = smallp.tile([P, 1], F32, tag="r2", name=f"r2_{qb}")
                nc.vector.reciprocal(r2, s2)
                # r2q = (r2 * s1) * lam fused in one DVE tensor_scalar
                r2q = smallp.tile([P, 1], F32, tag="r2q", name=f"r2q_{qb}")
                nc.vector.tensor_scalar(r2q, r2, s1, lam_sb, MUL, MUL)
                pend.append((qb, ets, r2q))

            def emit_combine_attn():
                qb, ets, r2q = pend.pop(0)
                # -p = e2*r2q - e1, one fused DVE pass -> bf16 for transposes
                pd = pdp.tile([P, SN, 512], BF16, tag="pd", name=f"pd_{qb}")
                nc.vector.scalar_tensor_tensor(
                    pd, ets[1][:, :, :], r2q, ets[0][:, :, :], MUL, SUB)
                # transpose -p into ptile[:, :, qb-block]; batch 8 transposes
                # per psum bank, evict with one strided copy
                for kc8 in range(NKC // 8):
                    tp = _tps.tile([P, 8, P], BF16, tag="tp",
                                   name=f"tp_{qb}_{kc8}")
                    for j in range(8):
                        kc = kc8 * 8 + j
                        nc.tensor.matmul(
                            tp[:, j], pd[:, kc // 4, ds((kc % 4) * P, P)],
                            ident, is_transpose=True)
                    nc.vector.tensor_copy(ptile[:, ts(kc8, 8), ts(qb, P)], tp)

            wT = wtp.tile([P, KD, QC], F32R, name="wT")

            def emit_wt_half(qh):
                # wT[d, qh-half] = sum_k x[k, d] * (-p)[q, k] ( = -(p@x)^T );
                # 256-wide q-halves so half 0 starts after combine(1) instead
                # of barriering on the full ptile
                for dh in range(2):
                    pw = wps.tile([P, 2, 512], F32, tag="ps",
                                  name=f"wt_{qh}_{dh}")
                    for j in range(2):
                        db = dh * 2 + j
                        for kc in range(NKC):
                            nc.tensor.matmul(
                                pw[:, j, :256], xn_sb[:, kc, ds(db * P, P)],
                                ptile[:, kc, ds(qh * 256, 256)],
                                start=(kc == 0), stop=(kc == NKC - 1))
                    nc.vector.tensor_copy(
                        wT[:, ds(dh * 2, 2), ds(qh * 256, 256)], pw[:, :, :256])

            def emit_outg(qb):
                # out = (p@x) @ W2 / s1: contract d, evict with scale -1/s1
                fo = fps.tile([P, DM], F32, tag="f", name=f"fo_{qb}")
                for dc in range(KD):
                    nc.tensor.matmul(fo, wT[:, dc, ts(qb, P)], w2_sb[:, dc],
                                     start=(dc == 0), stop=(dc == KD - 1))
                of = ofp.tile([P, DM], F32, tag="of", name=f"of_{qb}")
                nc.scalar.activation(of, fo, IDENT, scale=r1s[qb])
                nc.gpsimd.dma_start(out[ds(qb * P, P), :], of)

            # software pipeline: scores(qb+1) sits ahead of combine(qb) in the
            # PE queue so the PE never stalls on the ACT/DVE combine tail
            for qb in range(QB):
                e1t, s1t = emit_scores_half(qb, 0)
                e2t, s2t = emit_scores_half(qb, 1)
                emit_norms(qb, [e1t, e2t], [s1t, s2t])
                if qb > 0:
                    emit_combine_attn()
            emit_wt_half(0)
            emit_combine_attn()
            emit_outg(0)
            emit_outg(1)
            emit_wt_half(1)
            emit_outg(2)
            emit_outg(3)


def build_module(n_iters=1):
    nc = bacc.Bacc("TRN2", target_bir_lowering=False, debug=False)
    xT = nc.dram_tensor("xT", (D, S), F32R, kind="ExternalInput").ap()
    xn = nc.dram_tensor("xn", (S, D), BF16, kind="ExternalInput").ap()
    wqkv = nc.dram_tensor("wqkv", (D, 2 * D), F32R, kind="ExternalInput").ap()
    w2 = nc.dram_tensor("w2", (D, DM), F32R, kind="ExternalInput").ap()
    lam = nc.dram_tensor("lam", (P, 1), F32, kind="ExternalInput").ap()
    bq = nc.dram_tensor("bq", (P, MQ), F32, kind="ExternalInput").ap()
    out = nc.dram_tensor("out", (QC, DM), F32, kind="ExternalOutput").ap()
    with tile.TileContext(nc) as tc:
        for _ in range(n_iters):
            kernel_body(tc, xT, xn, wqkv, w2, lam, bq, out)
    nc.compile()
    return nc


_NC = None


def _get_module():
    global _NC
    if _NC is None:
        _NC = build_module()
    return _NC


def host_prep(**inputs):
    """Host-side input prep: returns (in_maps, lam, host_bias)."""
    x = np.asarray(inputs["x"], np.float32)
    Wqkv = np.asarray(inputs["Wqkv"], np.float32)
    bqkv = np.asarray(inputs["bqkv"], np.float32)
    Wv = np.asarray(inputs["Wv"], np.float32)
    bv = np.asarray(inputs["bv"], np.float32)
    Wo = np.asarray(inputs["Wo"], np.float32)
    bo = np.asarray(inputs["bo"], np.float32)
    lq1 = np.asarray(inputs["lq1"], np.float32)
    lk1 = np.asarray(inputs["lk1"], np.float32)
    lq2 = np.asarray(inputs["lq2"], np.float32)
    lk2 = np.asarray(inputs["lk2"], np.float32)

    lam = float(
        np.exp(np.sum(lq1 * lk1, dtype=np.float32))
        - np.exp(np.sum(lq2 * lk2, dtype=np.float32))
        + (LAMBDA_INIT - 0.6 * math.exp(-0.3 * LAYER_INDEX))
    )
    bq_host = np.ascontiguousarray(bqkv.reshape(MQ, P).T)
    lam_host = np.full((P, 1), lam, np.float32)
    # weight-only fold: out_proj absorbs the V projection
    W2 = np.ascontiguousarray(Wv @ Wo)

    in_maps = []
    for c in range(8):
        b, qs = divmod(c, NQ)
        # rotate the keys so this core's query slice sits at columns 0:512;
        # attention is permutation-invariant along keys (kT and V2 share the
        # rotated order), and the output rows are the original query slice
        xrot = np.roll(x[b], -qs * QC, axis=0)
        in_maps.append({
            "xT": np.ascontiguousarray(xrot.T),
            "xn": np.ascontiguousarray(xrot).astype(ml_dtypes.bfloat16),
            "wqkv": np.ascontiguousarray(Wqkv),
            "w2": W2,
            "lam": lam_host,
            "bq": bq_host,
        })
    # sum_k diff_attn[q, :] == 1 - lam exactly, so bv and bo fold into a
    # constant per-output-column correction.
    host_bias = ((1.0 - lam) * bv) @ Wo + bo
    return in_maps, lam, host_bias.astype(np.float32)


def kernel(**inputs):
    in_maps, _lam, host_bias = host_prep(**inputs)
    nc = _get_module()
    res = bass_utils.run_bass_kernel_spmd(nc, in_maps, core_ids=list(range(8)))
    out = np.empty((B, S, DM), np.float32)
    for c in range(8):
        b, qs = divmod(c, NQ)
        out[b, qs * QC : (qs + 1) * QC, :] = res.results[c]["out"]
    out += host_bias
    return out


# revision 55
# speedup vs baseline: 1.0522x; 1.0522x over previous
"""Differential attention kernel for Trainium2 (8 NeuronCores).

Key identities: everything after the differential combine is linear, so
    out = diff_attn @ (x @ Wv) @ Wo + bias == (diff_attn @ x) @ (Wv @ Wo) + bias
W2 = Wv @ Wo is a [512, 512] weight-only product folded on the host, which
removes the 8192-wide V projection / attn@V / out_proj entirely; and the
device contracts attention with x FIRST (p@x is S-wide, then a small @W2),
so no V-like projection of the full batch is ever materialized.

Sharding: 2 batches x 4 query-quarters (512 queries per core). Each core
computes full K for its batch plus Q for its own quarter, its 512x2048
slice of both attention maps, and the 512x512 output slice. Only the K
projection is duplicated within a batch group.

GEMMs run as float32r (fp32 storage, full-rate PE mode) except the
attention map / x-row operands of the p@x stage, which are bf16 (PE
transposes at 1.0 vs 1.5 cyc/row; 256-wide q-half streams at full rate).
Softmax is computed without max-subtraction (|logits| < ~12, safe in fp32).
The 1/sum(e1) normalizer is folded into the final PSUM eviction, so the
combine is a single fused DVE op computing -p = e2*(lam*s1/s2) - e1; the sign
is absorbed by evicting with scale = -1/s1. bqkv is applied via the ScalarE
bias port; bv/bo fold exactly on the host via sum_k(diff_attn[q,:]) == 1-lam.

Schedule notes (sim-tuned): scores use 2-bank psum tiles so each exp is
1024 wide (ACT instruction count gates the scores phase); combine of qb-1
is pipelined behind scores(qb); wT = -(p@x)^T runs in two q-halves so half 0
starts before the last combine finishes; the setup pool stays open through
the main phase so the next iteration's input DMAs prefetch mid-iteration;
output DMAs and the tail-only x/lam/bq loads ride the GpSimd queue so they
never head-of-line block the next iteration's input loads on the SP queue.
"""

import math

import ml_dtypes
import numpy as np

import concourse.bass as bass
from concourse import bacc
import concourse.mybir as mybir
import concourse.tile as tile
from concourse import bass_utils
from concourse.bass import ts, ds
from concourse.masks import make_identity

# Problem shapes (hardcoded per harness contract).
B = 2
S = 2048
D = 512
DM = 512             # output dim
P = 128
QC = 512             # queries per core
NQ = S // QC         # 4 q-shards per batch
SCALE = 1.0 / math.sqrt(64.0)
LAMBDA_INIT = 0.8
LAYER_INDEX = 0

F32 = mybir.dt.float32
F32R = mybir.dt.float32r
BF16 = mybir.dt.bfloat16
EXP = mybir.ActivationFunctionType.Exp
IDENT = mybir.ActivationFunctionType.Identity
AXX = mybir.AxisListType.X
MUL = mybir.AluOpType.mult
SUB = mybir.AluOpType.subtract

KD = D // P          # 4 contraction chunks of the input dim
MQ = (2 * D) // P    # 8 bias columns (Q: 0..3, K: 4..7)
SN = S // 512        # 4 key chunks of 512
NKC = S // P         # 16 key chunks of 128
QB = QC // P         # 4 q-blocks per core



def kernel_body(tc, xT, xn, wqkv, w2, lam, bq, out):
    nc = tc.nc
    # tolerate f32-typed dram tensors (e.g. run_kernel's sim harness)
    if xT.dtype != F32R:
        xT = xT.bitcast(F32R)
    if wqkv.dtype != F32R:
        wqkv = wqkv.bitcast(F32R)
    if w2.dtype != F32R:
        w2 = w2.bitcast(F32R)

    with tc.tile_pool(name="persist", bufs=1) as persist:
        kT = persist.tile([P, 4, S], F32R)      # K1,K2 transposed: [feat, k]
        qT = persist.tile([P, 4, QC], F32R)     # Q1,Q2 own slice:  [feat, q]
        xn_sb = persist.tile([P, NKC, D], BF16) # x rows: [k-in-block, kc, d]
        w2_sb = persist.tile([P, KD, DM], F32R)
        lam_sb = persist.tile([P, 1], F32)
        bq_sb = persist.tile([P, MQ], F32)
        ident_f32 = persist.tile([P, P], F32)
        ident = persist.tile([P, P], BF16)

        nc.gpsimd.dma_start(lam_sb, lam)
        nc.gpsimd.dma_start(bq_sb, bq)
        make_identity(nc, ident_f32)
        nc.vector.tensor_copy(ident, ident_f32)

        # ---------------- setup: Q, K, V2 projections ----------------
        # the setup pool stays open through the main phase so the main-loop
        # pools get disjoint SBUF; the next iteration's setup region then
        # frees mid-iteration and its input DMAs overlap this one's attention
        with (
            tc.tile_pool(name="setup", bufs=1) as setup,
        ):
            xTs = setup.tile([P, KD, S], F32R)
            wq_sb = setup.tile([P, KD, 2 * D], F32R)

            # loads in first-consumption order (Q proj, K proj, V2); own
            # q-slice is columns 0:512 (host rotates x per core)
            for dc in range(KD):
                nc.sync.dma_start(wq_sb[:, dc, :D], wqkv[ds(dc * P, P), :D])
                nc.sync.dma_start(xTs[:, dc, ts(0, 512)], xT[ds(dc * P, P), ts(0, 512)])
            for dc in range(KD):
                nc.sync.dma_start(wq_sb[:, dc, D:], wqkv[ds(dc * P, P), D:])
            for sn in range(1, SN):
                for dc in range(KD):
                    nc.sync.dma_start(xTs[:, dc, ts(sn, 512)],
                                      xT[ds(dc * P, P), ts(sn, 512)])
            for dc in range(KD):
                nc.sync.dma_start(w2_sb[:, dc], w2[ds(dc * P, P), :])
            for kc in range(NKC):
                nc.gpsimd.dma_start(xn_sb[:, kc], xn[ds(kc * P, P), :])

            with tc.tile_pool(name="spsum", bufs=8, space="PSUM") as spsum:
                # Q proj (own slice = keys 0:512 of the rotated batch)
                for m in range(4):
                    pt = spsum.tile([P, QC], F32, tag="ps")
                    for dc in range(KD):
                        nc.tensor.matmul(pt, wq_sb[:, dc, ts(m, P)],
                                         xTs[:, dc, ts(0, 512)],
                                         start=(dc == 0), stop=(dc == KD - 1))
                    nc.scalar.activation(qT[:, m], pt, IDENT,
                                         bias=bq_sb[:, m : m + 1])
                # K proj (full batch)
                for sn in range(SN):
                    for m in range(4):
                        pt = spsum.tile([P, 512], F32, tag="ps")
                        for dc in range(KD):
                            nc.tensor.matmul(pt, wq_sb[:, dc, ts(4 + m, P)],
                                             xTs[:, dc, ts(sn, 512)],
                                             start=(dc == 0), stop=(dc == KD - 1))
                        nc.scalar.activation(kT[:, m, ts(sn, 512)], pt, IDENT,
                                             bias=bq_sb[:, 4 + m : 5 + m])

            # ---------------- main: attention for own 512 queries ----------
            with (
                tc.tile_pool(name="e1p", bufs=2) as e1p,
                tc.tile_pool(name="pdp", bufs=2) as pdp,
                tc.tile_pool(name="e2p", bufs=2) as e2p,
                tc.tile_pool(name="smallp", bufs=3) as smallp,
                tc.tile_pool(name="r1p", bufs=QB) as r1p,
                tc.tile_pool(name="ptp", bufs=1) as ptp,
                tc.tile_pool(name="ofp", bufs=4) as ofp,
                tc.tile_pool(name="wtp", bufs=1) as wtp,
                tc.tile_pool(name="wps", bufs=2, space="PSUM") as wps,
                tc.tile_pool(name="tps", bufs=2, space="PSUM") as _tps,
                tc.tile_pool(name="fps", bufs=2, space="PSUM") as fps,
            ):
                _run_attention(nc, qT, kT, xn_sb, w2_sb, lam_sb, ident, out,
                               e1p, e2p, smallp, r1p, ptp, ofp, wps, _tps, fps, pdp,
                               wtp)


def _run_attention(nc, qT, kT, xn_sb, w2_sb, lam_sb, ident, out,
                   e1p, e2p, smallp, r1p, ptp, ofp, wps, _tps, fps, pdp,
                   wtp):
            ptile = ptp.tile([P, NKC, QC], BF16)
            r1s = [None] * QB
            pend = []

            def emit_scores_half(qb, mi):
                pool = e1p if mi == 0 else e2p
                et = pool.tile([P, SN, 512], BF16, tag=f"e{mi}",
                               name=f"e{mi}_{qb}")
                st = smallp.tile([P, SN // 2], F32, tag=f"sum{mi}",
                                 name=f"sum{mi}_{qb}")
                # 2-bank psum per pair of 512-chunks -> one 1024-wide exp
                # (halves the ACT instruction count, which gates this phase)
                for knw in range(SN // 2):
                    pt = wps.tile([P, 2, 512], F32, tag="ps",
                                  name=f"ps_{qb}_{mi}_{knw}")
                    for h in range(2):
                        kn = 2 * knw + h
                        for dc in range(2):
                            nc.tensor.matmul(
                                pt[:, h],
                                qT[:, 2 * mi + dc, ts(qb, P)],
                                kT[:, 2 * mi + dc, ts(kn, 512)],
                                start=(dc == 0), stop=(dc == 1))
                    nc.scalar.activation(
                        et[:, ds(2 * knw, 2)], pt, EXP, scale=SCALE,
                        accum_out=st[:, knw : knw + 1])
                return et, st

            def emit_norms(qb, ets, sums):
                # normalizers: final evict uses -1/s1; combine uses lam*s1/s2
                s1 = smallp.tile([P, 1], F32, tag="s1", name=f"s1_{qb}")
                nc.vector.reduce_sum(s1, sums[0], axis=AXX)
                s1n = smallp.tile([P, 1], F32, tag="s1n", name=f"s1n_{qb}")
                nc.vector.tensor_scalar_mul(s1n, s1, -1.0)
                r1n = r1p.tile([P, 1], F32, tag="r1", name=f"r1_{qb}")
                nc.vector.reciprocal(r1n, s1n)
                r1s[qb] = r1n
                s2 = smallp.tile([P, 1], F32, tag="s2", name=f"s2_{qb}")
                nc.vector.reduce_sum(s2, sums[1], axis=AXX)
                r2 = smallp.tile([P, 1], F32, tag="r2", name=f"r2_{qb}")
                nc.vector.reciprocal(r2, s2)
                # r2q = (r2 * s1) * lam fused in one DVE tensor_scalar
                r2q = smallp.tile([P, 1], F32, tag="r2q", name=f"r2q_{qb}")
                nc.vector.tensor_scalar(r2q, r2, s1, lam_sb, MUL, MUL)
                pend.append((qb, ets, r2q))

            def emit_combine_attn():
                qb, ets, r2q = pend.pop(0)
                # -p = e2*r2q - e1, one fused DVE pass -> bf16 for transposes
                pd = pdp.tile([P, SN, 512], BF16, tag="pd", name=f"pd_{qb}")
                nc.vector.scalar_tensor_tensor(
                    pd, ets[1][:, :, :], r2q, ets[0][:, :, :], MUL, SUB)
                # transpose -p into ptile[:, :, qb-block]; batch 8 transposes
                # per psum bank, evict with one strided copy
                for kc8 in range(NKC // 8):
                    tp = _tps.tile([P, 8, P], BF16, tag="tp",
                                   name=f"tp_{qb}_{kc8}")
                    for j in range(8):
                        kc = kc8 * 8 + j
                        nc.tensor.matmul(
                            tp[:, j], pd[:, kc // 4, ds((kc % 4) * P, P)],
                            ident, is_transpose=True)
                    nc.vector.tensor_copy(ptile[:, ts(kc8, 8), ts(qb, P)], tp)

            wT = wtp.tile([P, KD, QC], F32R, name="wT")

            def emit_wt_half(qh):
                # wT[d, qh-half] = sum_k x[k, d] * (-p)[q, k] ( = -(p@x)^T );
                # 256-wide q-halves so half 0 starts after combine(1) instead
                # of barriering on the full ptile
                for dh in range(2):
                    pw = wps.tile([P, 2, 512], F32, tag="ps",
                                  name=f"wt_{qh}_{dh}")
                    for j in range(2):
                        db = dh * 2 + j
                        for kc in range(NKC):
                            nc.tensor.matmul(
                                pw[:, j, :256], xn_sb[:, kc, ds(db * P, P)],
                                ptile[:, kc, ds(qh * 256, 256)],
                                start=(kc == 0), stop=(kc == NKC - 1))
                    nc.vector.tensor_copy(
                        wT[:, ds(dh * 2, 2), ds(qh * 256, 256)], pw[:, :, :256])

            def emit_outg(qb):
                # out = (p@x) @ W2 / s1: contract d, evict with scale -1/s1
                fo = fps.tile([P, DM], F32, tag="f", name=f"fo_{qb}")
                for dc in range(KD):
                    nc.tensor.matmul(fo, wT[:, dc, ts(qb, P)], w2_sb[:, dc],
                                     start=(dc == 0), stop=(dc == KD - 1))
                of = ofp.tile([P, DM], F32, tag="of", name=f"of_{qb}")
                nc.scalar.activation(of, fo, IDENT, scale=r1s[qb])
                nc.gpsimd.dma_start(out[ds(qb * P, P), :], of)

            # software pipeline: scores(qb+1) sits ahead of combine(qb) in the
            # PE queue so the PE never stalls on the ACT/DVE combine tail
            for qb in range(QB):
                e1t, s1t = emit_scores_half(qb, 0)
                e2t, s2t = emit_scores_half(qb, 1)
                emit_norms(qb, [e1t, e2t], [s1t, s2t])
                if qb > 0:
                    emit_combine_attn()
            emit_wt_half(0)
            emit_combine_attn()
            emit_outg(0)
            emit_outg(1)
            emit_wt_half(1)
            emit_outg(2)
            emit_outg(3)


def build_module(n_iters=1):
    nc = bacc.Bacc("TRN2", target_bir_lowering=False, debug=False)
    xT = nc.dram_tensor("xT", (D, S), F32R, kind="ExternalInput").ap()
    xn = nc.dram_tensor("xn", (S, D), BF16, kind="ExternalInput").ap()
    wqkv = nc.dram_tensor("wqkv", (D, 2 * D), F32R, kind="ExternalInput").ap()
    w2 = nc.dram_tensor("w2", (D, DM), F32R, kind="ExternalInput").ap()
    lam = nc.dram_tensor("lam", (P, 1), F32, kind="ExternalInput").ap()
    bq = nc.dram_tensor("bq", (P, MQ), F32, kind="ExternalInput").ap()
    out = nc.dram_tensor("out", (QC, DM), F32, kind="ExternalOutput").ap()
    with tile.TileContext(nc) as tc:
        for _ in range(n_iters):
            kernel_body(tc, xT, xn, wqkv, w2, lam, bq, out)
    nc.compile()
    return nc


_NC = None


def _get_module():
    global _NC
    if _NC is None:
        _NC = build_module()
    return _NC


def host_prep(**inputs):
    """Host-side input prep: returns (in_maps, lam, host_bias)."""
    x = np.asarray(inputs["x"], np.float32)
    Wqkv = np.asarray(inputs["Wqkv"], np.float32)
    bqkv = np.asarray(inputs["bqkv"], np.float32)
    Wv = np.asarray(inputs["Wv"], np.float32)
    bv = np.asarray(inputs["bv"], np.float32)
    Wo = np.asarray(inputs["Wo"], np.float32)
    bo = np.asarray(inputs["bo"], np.float32)
    lq1 = np.asarray(inputs["lq1"], np.float32)
    lk1 = np.asarray(inputs["lk1"], np.float32)
    lq2 = np.asarray(inputs["lq2"], np.float32)
    lk2 = np.asarray(inputs["lk2"], np.float32)

    lam = float(
        np.exp(np.sum(lq1 * lk1, dtype=np.float32))
        - np.exp(np.sum(lq2 * lk2, dtype=np.float32))
        + (LAMBDA_INIT - 0.6 * math.exp(-0.3 * LAYER_INDEX))
    )
    bq_host = np.ascontiguousarray(bqkv.reshape(MQ, P).T)
    lam_host = np.full((P, 1), lam, np.float32)
    # weight-only fold: out_proj absorbs the V projection
    W2 = np.ascontiguousarray(Wv @ Wo)

    in_maps = []
    for c in range(8):
        b, qs = divmod(c, NQ)
        # rotate the keys so this core's query slice sits at columns 0:512;
        # attention is permutation-invariant along keys (kT and V2 share the
        # rotated order), and the output rows are the original query slice
        xrot = np.roll(x[b], -qs * QC, axis=0)
        in_maps.append({
            "xT": np.ascontiguousarray(xrot.T),
            "xn": np.ascontiguousarray(xrot).astype(ml_dtypes.bfloat16),
            "wqkv": np.ascontiguousarray(Wqkv),
            "w2": W2,
            "lam": lam_host,
            "bq": bq_host,
        })
    # sum_k diff_attn[q, :] == 1 - lam exactly, so bv and bo fold into a
    # constant per-output-column correction.
    host_bias = ((1.0 - lam) * bv) @ Wo + bo
    return in_maps, lam, host_bias.astype(np.float32)


def kernel(**inputs):
    in_maps, _lam, host_bias = host_prep(**inputs)
    nc = _get_module()
    res = bass_utils.run_bass_kernel_spmd(nc, in_maps, core_ids=list(range(8)))
    out = np.empty((B, S, DM), np.float32)
    for c in range(8):
        b, qs = divmod(c, NQ)
        out[b, qs * QC : (qs + 1) * QC, :] = res.results[c]["out"]
    out += host_bias
    return out
